# revision 67
# baseline (speedup 1.0000x reference)
"""Multi-head causal attention (B=2, T=2048, E=1024, H=16, D=64) on 8 trn2 cores.

Sharding: tensor-parallel over heads — core c owns heads {2c, 2c+1} (a 128-wide
slice of the hidden dim). Each core computes q/k/v projections for its heads
over the full sequence, causal attention, and a partial output projection
(contraction over its 128 rows of Wo). The host sums the 8 partials + bias.

Per-core device program (SPMD — one NEFF, different weight slices per core):
  projections run as split-fp8 DoubleRow matmuls: x = xh + xl and W*32 =
    Wh + Wl quantized to e4m3 on host; q' = xh@Wh + xl@Wh + xh@Wl = 32*q
    to ~1.3e-3 rel err (better than bf16) at 0.75x the bf16 PE cost
    (12 DoubleRow passes of K=256 vs 8 bf16 passes of K=128). The 32x
    scale rides through attention (1/Z cancels it) and is divided out on
    the host after the partial-sum reduction.
  schedule: attention starts as soon as the half-0 q/k/v units land;
    all remaining projection units (both batches) are interleaved one per
    attention wave via an explicit feed table, so the exp-bound wave
    pipeline absorbs the projection PE time. Each chunk's normalize +
    output-projection epilogue is software-pipelined into the next
    chunk's waves (normalize right after the next chunk's first exp,
    oproj steps two per wave) so the PE never stalls on the
    recip->broadcast->normalize chain at chunk boundaries.
  attention, per (batch, 512-wide tq chunk), in waves of two 128-row tk
    blocks: S^T = K Q^T per head in bf16 (fp8 scores fail the 2e-2 gate);
    P^T = exp(S^T * 0.125/1024) on ScalarE straight out of PSUM with
    leading causally-dead columns trimmed; diagonal 128x128 triangle
    masks multiplied into P^T on GpSimd (SBUF-only, keeps DVE free);
    O^T|Z = [V|1]^T P^T with causality-limited ranges; 1/Z (fp16)
    broadcast across the 64 head dims by a K=1 outer-product matmul;
    normalize on VectorE.
  output: out[tq, :] = O^T.T @ Wo_slice (K=128), DVE copies to bf16 +
    DMA per 128 rows (the last chunk splits copies DVE/ScalarE — exp is
    done by then). ScalarE otherwise runs exp only.
"""

import os
import numpy as np
import ml_dtypes

import concourse.bass as bass
import concourse.tile as tile
from concourse import bacc, mybir
from concourse.bass_utils import run_bass_kernel_spmd
from contextlib import ExitStack

B, T, E, H, D = 2, 2048, 1024, 16, 64
BT = B * T            # 4096 tokens total
NCORE = 8
KC = E // 128         # contraction chunks for projections = 8
KP = KC // 2          # DoubleRow kc-pairs = 4
CQ = 512              # tq chunk width
NQB = T // CQ         # tq chunks per batch = 4
NKB = T // 128        # tk blocks per batch = 16

F32 = mybir.dt.float32
BF16 = mybir.dt.bfloat16
F8 = mybir.dt.float8e4
AF = mybir.ActivationFunctionType
DR = mybir.MatmulPerfMode.DoubleRow

_cache = {}


def _build():
    nc = bacc.Bacc("TRN2", target_bir_lowering=False, debug=False,
                   num_devices=NCORE)

    # x hi/lo fp8 splits, host-arranged [kp, p, hl, kc_in_pair, t] so one
    # DMA per (kc-pair, 512-token half) brings both splits and the
    # [hl, kc, t] free dims collapse within the 3-dim DMA AP limit.
    xhl = nc.dram_tensor("xhl", [KP, 128, 4 * BT], F8,
                         kind="ExternalInput").ap()
    # all six fp8 weight slices stacked: row = s*128 + p, s in
    # (wqh, wql, wkh, wkl, wvh, wvl)
    w6 = nc.dram_tensor("w6", [6 * 128, E], F8, kind="ExternalInput").ap()
    wo = nc.dram_tensor("wo", [128, E], BF16, kind="ExternalInput").ap()
    tri = nc.dram_tensor("tri", [128, 128], BF16, kind="ExternalInput").ap()
    iden = nc.dram_tensor("iden", [128, 128], BF16, kind="ExternalInput").ap()
    out = nc.dram_tensor("out", [BT, E], BF16, kind="ExternalOutput").ap()

    with tile.TileContext(nc) as tc, ExitStack() as ctx:
        pers = ctx.enter_context(tc.tile_pool(name="pers", bufs=1))

        # fp8 weight slices, [128p, slot, kc, 128d]: DR passes use kc-pairs.
        w6_sb = pers.tile([128, 6, KC, 128], F8, tag="w6")
        wsb = {n: w6_sb[:, i]
               for i, n in enumerate(("wqh", "wql", "wkh", "wkl",
                                      "wvh", "wvl"))}
        wo_sb = pers.tile([128, E], BF16, tag="wo")
        tri_sb = pers.tile([128, 128], BF16, tag="tri")
        iden_sb = pers.tile([128, 128], BF16, tag="iden")
        zeros_sb = pers.tile([128, 512], BF16, tag="zeros")
        ones_sb = pers.tile([128, 64], mybir.dt.float16, tag="ones")
        qt_sb = pers.tile([128, BT], BF16, tag="qt")    # [dims(2 heads), tok]
        kt_sb = pers.tile([128, BT], BF16, tag="kt")
        # V natural + ones col per head: [tok%128, blk, h, d|1]
        v_sb = pers.tile([128, BT // 128, 2, 65], BF16, tag="v")
        ot_sb = pers.tile([128, BT], BF16, tag="ot")    # attn out, [dims, tok]

        # q/k weights land first (the v slices aren't needed until the
        # third unit); tri/wo (first diag mask / oproj) go after the first
        # pair's x DMAs.
        w6r = w6.rearrange("(s p) (kc d) -> p s kc d", s=6, kc=KC)
        nc.sync.dma_start(w6_sb[:, 0:4], w6r[:, 0:4])
        nc.sync.dma_start(w6_sb[:, 4:6], w6r[:, 4:6])
        nc.vector.memset(ones_sb[:], 1.0)
        nc.vector.memset(zeros_sb[:], 0.0)
        nc.vector.memset(v_sb[:, :, :, 64:65], 1.0)

        def load_late_weights():
            nc.sync.dma_start(tri_sb[:], tri[:])
            nc.sync.dma_start(wo_sb[:], wo[:])
            nc.sync.dma_start(iden_sb[:], iden[:])

        xts_pool = ctx.enter_context(tc.tile_pool(name="xts", bufs=24))
        sc_pool = ctx.enter_context(tc.tile_pool(name="sc", bufs=2, space="PSUM"))
        pv_pool = ctx.enter_context(tc.tile_pool(name="pv", bufs=2, space="PSUM"))
        # zb/oproj psums and the projection psums share one 2-slot pool so
        # wave score slots are never stolen by interleaved projections.
        ops_pool = ctx.enter_context(tc.tile_pool(name="ops", bufs=2, space="PSUM"))
        pt_pool = ctx.enter_context(tc.tile_pool(name="pt", bufs=3))
        zr_pool = ctx.enter_context(tc.tile_pool(name="zr", bufs=3))
        on_pool = ctx.enter_context(tc.tile_pool(name="on", bufs=3))
        ost_pool = ctx.enter_context(tc.tile_pool(name="ost", bufs=6))

        def proj_pair_units(t0, hf_order=(0, 1)):
                # x chunks [128, hl, kc_in_pair, 512t], one DMA per
                # (kc-pair, 512-token half); the half feeding the sooner-
                # consumed units goes first on the queue.
                xts = [[None, None] for _ in range(KP)]
                for hf in hf_order:
                    for kp in range(KP):
                        xt = xts_pool.tile([128, 2, 2, CQ], F8, tag="xt",
                                           name=f"xt_{t0}_{hf}_{kp}")
                        nc.sync.dma_start(
                            xt[:],
                            xhl[kp].rearrange(
                                "p (hl kc t) -> p hl kc t", hl=2, kc=2)
                                [:, :, :,
                                 (t0 + hf) * CQ:(t0 + hf + 1) * CQ])
                        xts[kp][hf] = xt

                # split3 terms: xh@Wh + xh@Wl + xl@Wh (equal-weight psum)
                def qk_unit(whn, wln, dst_sb, hf):
                    t_ = t0 + hf
                    def emit():
                        ps = ops_pool.tile([128, CQ], F32, tag="o",
                                           name=f"qkps{t_}_{whn}")
                        terms = [(0, wsb[whn]), (0, wsb[wln]), (1, wsb[whn])]
                        n = 0
                        for xi, wt in terms:
                            for kp in range(KP):
                                nc.tensor.matmul(
                                    ps[:], wt[:, 2 * kp:2 * kp + 2],
                                    xts[kp][hf][:, xi],
                                    start=(n == 0), stop=(n == 3 * KP - 1),
                                    perf_mode=DR)
                                n += 1
                        nc.vector.tensor_copy(
                            dst_sb[:, t_ * CQ:(t_ + 1) * CQ], ps[:])
                    return emit

                def v_unit(hf):
                    t_ = t0 + hf
                    def emit():
                        v_ps = ops_pool.tile([128, CQ], F32, tag="o",
                                             name=f"vps{t_}")
                        for j in range(CQ // 128):
                            jf = j * 128
                            terms = [(0, wsb["wvh"]), (0, wsb["wvl"]),
                                     (1, wsb["wvh"])]
                            n = 0
                            for xi, wt in terms:
                                for kp in range(KP):
                                    nc.tensor.matmul(
                                        v_ps[:, j * 128:(j + 1) * 128],
                                        xts[kp][hf][:, xi, :, jf:jf + 128],
                                        wt[:, 2 * kp:2 * kp + 2],
                                        start=(n == 0),
                                        stop=(n == 3 * KP - 1),
                                        perf_mode=DR)
                                    n += 1
                        b4 = t_ * (CQ // 128)
                        nc.vector.tensor_copy(
                            v_sb[:, b4:b4 + 4, :, 0:64],
                            v_ps[:].rearrange("p (j h v) -> p j h v",
                                              j=4, h=2))
                    return emit

                return [qk_unit("wqh", "wql", qt_sb, 0),
                        qk_unit("wkh", "wkl", kt_sb, 0),
                        v_unit(0),
                        qk_unit("wqh", "wql", qt_sb, 1),
                        qk_unit("wkh", "wkl", kt_sb, 1),
                        v_unit(1)]

        EXP_SCALE = float(D) ** -0.5 / 1024.0  # q,k carry a 32x scale each

        # Projection units interleave into attention waves: half h's q/k/v
        # units are pair_units[h // 2][3 * (h % 2) + {0,1,2}]. The feed
        # table places each unit (and each later pair's DMA batch) at the
        # earliest wave whose PE slack absorbs it while still completing
        # before its first consumer wave.
        pair_units = {}

        def mk_pair(pi):
            # pair 3's half-1 units (q7/k7/v7) feed the early b1 chunk:
            # their x tiles go first on the DMA queue.
            pair_units[pi] = proj_pair_units(
                2 * pi, hf_order=(1, 0) if pi == 3 else (0, 1))

        def unit(h, j):  # j: 0=q, 1=k, 2=v
            pair_units[h // 2][3 * (h % 2) + j]()

        # PE p-state warmup: the tensor engine needs ~3us of continuous
        # execution to reach full clock. Burn the initial DMA-wait window
        # on dummy matmuls so the first projections run at speed.
        warm = ops_pool.tile([128, 512], F32, tag="o", name="warm")
        for _ in range(50):
            nc.tensor.matmul(warm[0:64, 0:64], ones_sb[0:64, :],
                             ones_sb[0:64, :], start=True, stop=True)

        mk_pair(0)
        load_late_weights()
        mk_pair(1)
        unit(0, 0)  # q half0
        unit(0, 1)  # k half0
        unit(0, 2)  # v half0 (cq0-w0's PV reads it: must precede the wave)
        # Chunks from both batches interleave so exp-heavy b1 chunks sit
        # next to feed-rich b0 chunks and neither engine starves; the
        # shortest chunk (b1 cq0) lands last to minimize the tail.
        chunks = [(0, 0), (0, 1), (0, 2), (1, 3), (0, 3),
                  (1, 2), (1, 1), (1, 0)]
        feed = {  # (chunk_idx, wave) -> list of callables
            (0, 0): [lambda: unit(1, 0)],
            (0, 1): [lambda: unit(1, 1)],
            (1, 0): [lambda: unit(1, 2), lambda: mk_pair(2)],
            (1, 1): [lambda: unit(2, 0)],
            (1, 2): [lambda: unit(2, 1)],
            (1, 3): [lambda: unit(2, 2)],
            (2, 0): [lambda: mk_pair(3), lambda: unit(3, 0)],
            (2, 1): [lambda: unit(3, 1)],
            (2, 2): [lambda: unit(3, 2)],
            (2, 4): [lambda: unit(7, 0), lambda: unit(4, 1)],
            (2, 5): [lambda: unit(4, 2)],
            (3, 0): [lambda: unit(5, 1)],
            (3, 1): [lambda: unit(5, 2)],
            (3, 2): [lambda: unit(6, 1)],
            (3, 3): [lambda: unit(6, 2)],
            (3, 4): [lambda: unit(7, 1)],
            (3, 5): [lambda: unit(7, 2)],
            (3, 6): [lambda: unit(6, 0)],
            (3, 7): [lambda: unit(5, 0)],
            (4, 0): [lambda: unit(4, 0)],
        }

        pending_norm = [None]   # normalize closure of the previous chunk
        pending_oproj = []      # oproj j-step closures of previous chunks

        for ci, (b, cq) in enumerate(chunks):
            if True:
                tb = b * T  # token offset of this batch
                if True:
                    icq = ci
                    tq0 = cq * CQ
                    nblk = (tq0 + CQ) // 128  # causal: tk blocks needed
                    pt = [pt_pool.tile([128, NKB, CQ], BF16, tag=f"pt{h}",
                                       name=f"pt{h}_{b}_{cq}")
                          for h in range(2)]
                    # O|Z accumulators in [tq, d] layout: [128tq, j, d|1].
                    # All four j accumulation groups share one bank, so a
                    # single zeroing matmul opens the bank (the sim's
                    # pending-zero marking is bank-wide: per-j start=True
                    # flags would clobber sibling groups).
                    pv = [pv_pool.tile([128, 4, 128], F32, tag="pv",
                                       name=f"pv{h}_{b}_{cq}")
                          for h in range(2)]
                    for h in range(2):
                        nc.tensor.matmul(
                            pv[h][:, :, 0:65], zeros_sb[:, 0:128],
                            zeros_sb[:, 0:260],
                            start=True, stop=False, skip_group_check=True)

                    for w in range(nblk // 2):  # waves of 2 tk blocks
                        kbs = (2 * w, 2 * w + 1)
                        sc = [sc_pool.tile([128, 2 * CQ], F32, tag="sc",
                                           name=f"sc{h}_{b}_{cq}_{w}")
                              for h in range(2)]
                        for i, kb in enumerate(kbs):
                            tk0 = kb * 128
                            f0 = max(tk0 - tq0, 0)
                            for h in range(2):
                                hs = slice(h * 64, (h + 1) * 64)
                                nc.tensor.matmul(
                                    sc[h][:, i * CQ + f0:(i + 1) * CQ],
                                    kt_sb[hs, tb + tk0:tb + tk0 + 128],
                                    qt_sb[hs, tb + tq0 + f0:tb + tq0 + CQ],
                                    start=True, stop=True)
                        # exp: trim causally-dead columns. On the deepest
                        # diagonal wave, per-block exps skip the second
                        # block's dead span too.
                        fw = max(2 * w * 128 - tq0, 0)
                        fw2 = max((2 * w + 1) * 128 - tq0, 0)
                        for h in range(2):
                            ptf = pt[h].rearrange("p a b -> p (a b)")
                            w0c = 2 * w * CQ
                            if fw >= 256:
                                nc.scalar.activation(
                                    ptf[:, w0c + fw:w0c + CQ],
                                    sc[h][:, fw:CQ],
                                    AF.Exp, scale=EXP_SCALE)
                                nc.scalar.activation(
                                    ptf[:, w0c + CQ + fw2:w0c + 2 * CQ],
                                    sc[h][:, CQ + fw2:],
                                    AF.Exp, scale=EXP_SCALE)
                            else:
                                nc.scalar.activation(
                                    ptf[:, w0c + fw:w0c + 2 * CQ],
                                    sc[h][:, fw:],
                                    AF.Exp, scale=EXP_SCALE)
                        # previous chunk's normalize: its PE work (the 1/Z
                        # broadcast) waits on a DVE recip; inject it here so
                        # the wait hides under this wave's exp.
                        if w == 0 and pending_norm[0] is not None:
                            pending_norm[0]()
                            pending_norm[0] = None
                        for i, kb in enumerate(kbs):
                            tk0 = kb * 128
                            s = tk0 - tq0
                            for h in range(2):
                                if 0 <= s < CQ:  # diagonal: triangle mask
                                    nc.gpsimd.tensor_mul(
                                        pt[h][:, kb, s:s + 128],
                                        pt[h][:, kb, s:s + 128], tri_sb[:])
                                # P^T block as stationary, [V|1] moving:
                                # 65 moving rows per (block, tq-128-chunk)
                                # instead of up-to-512 per block.
                                for j in range(max(0, kb - 4 * cq), 4):
                                    nc.tensor.matmul(
                                        pv[h][:, j, 0:65],
                                        pt[h][:, kb,
                                              j * 128:(j + 1) * 128],
                                        v_sb[:, b * NKB + kb, h],
                                        start=False,
                                        stop=(kb == 4 * cq + j),
                                        skip_group_check=True)

                        for fu in feed.get((ci, w), ()):
                            fu()
                        for _ in range(2):  # drain prev chunks' oproj steps
                            if pending_oproj:
                                pending_oproj.pop(0)()

                    def make_norm(pv=pv, b=b, cq=cq, ci=ci, tb=tb,
                                  tq0=tq0):
                        def norm():
                            # Z sits in column 64 of each [tq, j] row:
                            # 1/Z is a per-partition scalar, so normalize
                            # is a tensor_scalar per (h, j); then PE
                            # transposes O[tq, d] back to O^T for the
                            # output projection (both heads share one
                            # [128, 128] psum tile — disjoint partitions).
                            zrn = zr_pool.tile([128, 2, 4], F32, tag="zr",
                                               name=f"zr_{b}_{cq}")
                            for h in range(2):
                                nc.vector.reciprocal(
                                    zrn[:, h],
                                    pv[h][:, :, 64:65].rearrange(
                                        "p j o -> p (j o)"))
                            for j in range(4):
                                o_n = on_pool.tile(
                                    [128, 2, 64], BF16, tag="on",
                                    name=f"on_{b}_{cq}_{j}")
                                tp = ops_pool.tile(
                                    [128, 128], BF16, tag="o",
                                    name=f"tp_{b}_{cq}_{j}")
                                for h in range(2):
                                    # final chunk: ScalarE is free after
                                    # the last exp — run the normalize
                                    # scale there to shorten the tail
                                    if ci == len(chunks) - 1:
                                        nc.scalar.activation(
                                            o_n[:, h],
                                            pv[h][:, j, 0:64],
                                            AF.Copy,
                                            scale=zrn[:, h, j:j + 1])
                                    else:
                                        nc.vector.tensor_scalar_mul(
                                            o_n[:, h], pv[h][:, j, 0:64],
                                            zrn[:, h, j:j + 1])
                                # one transpose covers both heads: the
                                # [tq, (h d)] free layout transposes to
                                # exactly ot's row order
                                nc.tensor.transpose(
                                    tp[:],
                                    o_n.rearrange("p h d -> p (h d)"),
                                    iden_sb[:])
                                nc.vector.tensor_copy(
                                    ot_sb[:, tb + tq0 + j * 128:
                                          tb + tq0 + (j + 1) * 128],
                                    tp[:])
                        return norm

                    def make_oproj(j, b=b, cq=cq, ci=ci, tb=tb, tq0=tq0):
                        last_cq = (ci == len(chunks) - 1)
                        def oproj():
                            tqg = tb + tq0 + j * 128
                            ost = ost_pool.tile([128, 1024], BF16,
                                                tag="ost",
                                                name=f"ost_{b}_{cq}_{j}")
                            for eh in range(2):
                                fin_pool, fin_tag = ((sc_pool, "sc")
                                                     if last_cq
                                                     else (ops_pool, "o"))
                                o_ps = fin_pool.tile(
                                    [128, 512], F32, tag=fin_tag,
                                    name=f"o_{b}_{cq}_{j}_{eh}")
                                nc.tensor.matmul(
                                    o_ps[:], ot_sb[:, tqg:tqg + 128],
                                    wo_sb[:, eh * 512:(eh + 1) * 512],
                                    start=True, stop=True)
                                dst = ost[:, eh * 512:(eh + 1) * 512]
                                # the final chunk's eh==1 copies ride on
                                # ScalarE (exp is done by then)
                                if eh == 1 and last_cq:
                                    nc.scalar.copy(dst, o_ps[:])
                                else:
                                    nc.vector.tensor_copy(dst, o_ps[:])
                            nc.sync.dma_start(out[tqg:tqg + 128, :],
                                              ost[:])
                        return oproj

                    if ci == len(chunks) - 1:  # final chunk: emit directly
                        make_norm()()
                        for j in range(CQ // 128):
                            make_oproj(j)()
                    else:
                        pending_norm[0] = make_norm()
                        pending_oproj.extend(
                            make_oproj(j) for j in range(CQ // 128))

    nc.compile()
    return nc


def _host_prep(x, Wq, Wk, Wv, Wo):
    bf = ml_dtypes.bfloat16
    f8 = ml_dtypes.float8_e4m3
    xT = np.ascontiguousarray(
        np.asarray(x, dtype=np.float32).reshape(BT, E).T)
    xh = xT.astype(f8)
    xlo = (xT - xh.astype(np.float32)).astype(f8)
    # [kp, p, hl, kc_in_pair, t] flattened to [KP, 128, 4*BT]
    stacked = np.stack([xh.reshape(KC, 128, BT),
                        xlo.reshape(KC, 128, BT)], axis=1)  # kc hl p t
    xhl = np.ascontiguousarray(
        stacked.reshape(KP, 2, 2, 128, BT)        # kp kcin hl p t
               .transpose(0, 3, 2, 1, 4)          # kp p hl kcin t
               .reshape(KP, 128, 4 * BT))

    # tri[p, f] = 1 where kept (f >= p), applied to the diagonal 128x128
    # sub-block of P^T (tk on partitions, tq on free)
    p = np.arange(128)[:, None]
    f = np.arange(128)[None, :]
    tri = (f >= p).astype(bf)

    def perm(w):
        # [E, 128] -> [128p, kc, 128d] flattened: w[kc*128+p, d] -> out[p, kc, d]
        return np.ascontiguousarray(
            w.reshape(KC, 128, 128).transpose(1, 0, 2).reshape(128, E))

    def split8(w32):
        hi = w32.astype(f8)
        lo = (w32 - hi.astype(np.float32)).astype(f8)
        return hi, lo

    Wq = np.asarray(Wq, dtype=np.float32)
    Wk = np.asarray(Wk, dtype=np.float32)
    Wv = np.asarray(Wv, dtype=np.float32)
    Wo = np.asarray(Wo, dtype=np.float32)

    in_maps = []
    for c in range(NCORE):
        sl = slice(c * 128, (c + 1) * 128)
        parts = []
        for W in (Wq, Wk, Wv):
            hi, lo = split8(perm(W[:, sl] * 32.0))
            parts += [hi, lo]
        m = {"xhl": xhl, "tri": tri,
             "iden": np.eye(128, dtype=bf),
             "w6": np.ascontiguousarray(np.concatenate(parts, axis=0)),
             "wo": np.ascontiguousarray(Wo[sl, :]).astype(bf)}
        in_maps.append(m)
    return in_maps


def kernel(x, Wq, Wk, Wv, Wo, bo, _trace=False, _trace_kwargs=None):
    if "nc" not in _cache:
        _cache["nc"] = _build()
    nc = _cache["nc"]

    in_maps = _host_prep(x, Wq, Wk, Wv, Wo)
    kw = {}
    if _trace:
        kw = dict(trace=True, trace_cores=[0], **(_trace_kwargs or {}))
    res = run_bass_kernel_spmd(nc, in_maps, core_ids=list(range(NCORE)), **kw)
    _cache["last_result"] = res

    total = np.zeros((BT, E), dtype=np.float32)
    for r in res.results:
        total += np.asarray(r["out"], dtype=np.float32)
    total *= 1.0 / 32.0  # q,k,v carried a 32x host prescale; 1/Z cancels one
    total += np.asarray(bo, dtype=np.float32)[None, :]
    return total.reshape(B, T, E)
